# revision 11
# baseline (speedup 1.0000x reference)
"""Trainium2 Bass kernel for nn_DLFG_79817672229311 (segment_reduce).

Computes, data-parallel over the batch axis on 8 NeuronCores:
  history = [extInfo, ratings]                       # [BS, 20032] per core
  x1 = lrelu(history @ w1.T + b1); BN eval           # BN folded into w2 on host
  x2..x5 = lrelu(x @ wl.T + bl)
  gen = tanh(x5 @ w6.T + b6)                         # [BS, 65]
  s, cnt = per-row sum / count of nonzero ratings
  meanV = s / cnt                                    # cnt >= MIN_CNT always holds here
  out = gen[:, :64] @ movie_factors.T + (meanV + gen[:, 64])[:, None] + movie_bias

Design (per core), v2:
- Activations ride transposed ([feature, batch]): batch (512) is the matmul
  free dim, features the partition dim, so no on-device transposes are needed.
- History is staged to SBUF once in fp8 (ratings 0..5 are exact in e4m3) and
  layer 1 runs fp8 DoubleRow against 2^15-pre-scaled fp8 w1 slabs streamed
  from HBM; layers 2-5 are fp8 DoubleRow with per-layer pre-scales; layer 6
  stays bf16.  LeakyReLU is the native Lrelu ACT op, alternated with a 2-op
  DVE decomposition so the layer-1 epilogue never serializes on one engine.
- A chain of dummy matmuls at t=0 ramps the PE out of its low p-state while
  the first history/w1 DMAs (issued before everything else, split across
  queues) are still in flight.
- s is a PE ones-matmul chain over the resident history (DoubleRow,
  [128,2,16] ones to satisfy the pair-step%16 ISA rule); cnt is a 4-wide
  fused min+add chain on the otherwise-idle Vector engine.  Both finish
  under the layer-1 shadow.
- meanV + gen_last is folded into the reconstruction matmul itself: an extra
  genext row carries addv against a constant 16.0 row in mft, so the recon
  epilogue is a bias-free x/16 copy (alternating DVE/ACT) and no transpose
  bounce through DRAM is needed.  movie_factors ride fp8 x16 in a 66-row
  tile; output is written fp16 (host upcasts).
- The MLP and reconstruction are pipelined per 128-row batch tile so output
  DMA (the tail phase's real floor) starts right after layer 1 + s-chain.
"""

import math
import sys

sys.path.insert(0, "/opt/trn_rl_repo")

import numpy as np
import ml_dtypes

BF16 = ml_dtypes.bfloat16
FP8 = ml_dtypes.float8_e4m3

NCORES = 8
BN_EPS = 0.05
SLOPE = 0.01

FULL_CFG = dict(
    BS=512,  # per-core batch
    UINFO=32,
    M=20000,
    F=64,
    DIMS=(1024, 512, 256, 512, 1024, 65),  # fan-outs of the 6 linear layers
    HTC=4,  # history K-tiles per DMA chunk (must be even for DoubleRow pairs)
    MCHUNK=512,  # reconstruction movie-axis chunk (one PSUM bank)
    W1_SCALE=2.0**15,  # fp8 pre-scale: w1 ~ U(+-0.007) sits in e4m3 subnormals
    OUT_FP16=True,  # write output fp16 (host upcasts) to halve the drain DMA
    MLP_SCALES=(4096.0, 4096.0, 2048.0, 4096.0),  # 2^k per layer, keep |w|*s under e4m3 max 240
    WARMUP_N=44,  # dummy matmuls to ramp the PE p-state during initial DMA wait
    OST_BUFS=12,  # recon output staging depth
    EPI_DVE=True,  # alternate layer epilogues between ACT and DVE
    S_DVE_TILES=0,  # history tiles whose s-sum runs on DVE instead of the PE
    S_GPSIMD=False,  # run the whole s-chain on GpSimd (frees the PE entirely)
)


def _derived(cfg):
    d = dict(cfg)
    d["KH"] = cfg["UINFO"] + cfg["M"]
    d["T1"] = math.ceil(d["KH"] / 128)  # history K tiles (padded)
    d["NBT"] = cfg["BS"] // 128  # batch tiles per core
    mch = cfg["MCHUNK"]
    d["CHUNKS"] = [(o, min(mch, cfg["M"] - o)) for o in range(0, cfg["M"], mch)]
    d["NHC"] = math.ceil(d["T1"] / cfg["HTC"])  # history DMA chunks
    return d


def build_nc(cfg):
    """Build + compile the (single-core SPMD) Bass program."""
    import concourse.bass as bass
    import concourse.tile as tile
    from concourse import bacc, mybir

    d = _derived(cfg)
    BS, UINFO, M, F = cfg["BS"], cfg["UINFO"], cfg["M"], cfg["F"]
    DIMS = cfg["DIMS"]
    T1, NBT, CHUNKS, HTC = d["T1"], d["NBT"], d["CHUNKS"], cfg["HTC"]
    FO1 = DIMS[0]
    FO1T = FO1 // 128
    w1_unscale = 1.0 / cfg["W1_SCALE"]
    f32 = mybir.dt.float32
    bf16 = mybir.dt.bfloat16
    f16 = mybir.dt.float16
    f8 = mybir.dt.float8e4
    AF = mybir.ActivationFunctionType
    ALU = mybir.AluOpType

    nc = bacc.Bacc("TRN2", target_bir_lowering=False, debug=False)

    # ---- DRAM I/O ----
    ht_d = nc.dram_tensor("ht", [128, T1, BS], f8, kind="ExternalInput")
    w1t_d = nc.dram_tensor("w1t", [128, T1, FO1], f8, kind="ExternalInput")
    w_d = {}
    for li in range(2, 7):
        fi, fo = DIMS[li - 2], DIMS[li - 1]
        wdt = f8 if li < 6 else bf16
        w_d[li] = nc.dram_tensor(f"w{li}t", [128, fi // 128, fo], wdt, kind="ExternalInput")
    bp_d = {}
    for li in range(1, 6):
        fot = math.ceil(DIMS[li - 1] / 128)
        bp_d[li] = nc.dram_tensor(f"b{li}p", [128, fot], f32, kind="ExternalInput")
    b6_d = nc.dram_tensor("b6p", [128, 1], f32, kind="ExternalInput")
    mft_d = nc.dram_tensor("mft", [66, M], f8, kind="ExternalInput")
    odt = f16 if cfg.get("OUT_FP16") else bf16
    out_d = nc.dram_tensor("out", [BS, M], odt, kind="ExternalOutput")

    # layer-1 history segments: small warmup segs so the first matmuls
    # start ASAP, then HTC-sized chunks
    segs = []
    t0 = 0
    for tn in ([2, 2] if T1 > 4 else []):
        segs.append((t0, tn))
        t0 += tn
    while t0 < T1:
        tn = min(HTC, T1 - t0)
        segs.append((t0, tn))
        t0 += tn
    NSEG = len(segs)

    with tile.TileContext(nc) as tc, bass.ExitStack() as ctx:
        const = ctx.enter_context(tc.tile_pool(name="const", bufs=1))
        htp = ctx.enter_context(tc.tile_pool(name="htp", bufs=1))
        w1p = ctx.enter_context(tc.tile_pool(name="w1p", bufs=8))
        actp = ctx.enter_context(tc.tile_pool(name="actp", bufs=1))
        scr = ctx.enter_context(tc.tile_pool(name="scr", bufs=4))
        ost = ctx.enter_context(tc.tile_pool(name="ost", bufs=cfg["OST_BUFS"]))
        psp = ctx.enter_context(tc.tile_pool(name="psp", bufs=8, space="PSUM"))

        # ---- PE p-state warmup: dummy matmuls on a zeroed tile while the
        # first real DMAs land.  The TRN2 PE ramps 0.65->1.2->2.4 GHz with
        # ~3us of continuous execution; without this the first ~4 layer-1
        # steps run at less than half speed.
        warm = const.tile([128, 128], f8, name="warm", tag="warm")
        nc.vector.memset(warm[:], 0.0)
        wps = psp.tile([128, 128], f32, name="wps", tag="ps")
        for _ in range(cfg["WARMUP_N"]):
            nc.tensor.matmul(wps[:], lhsT=warm[:], rhs=warm[:], start=True, stop=True)

        # ---- first history segment + first w1 slab, issued before all other
        # DMAs and split across queues so the first real matmul fires early.
        ht_tiles = {}
        ts0, tn0 = segs[0]
        htt0 = htp.tile([128, tn0, BS], f8, name="ht", tag="ht", bufs=NSEG)
        for j in range(tn0):
            nc.sync.dma_start(out=htt0[:, j : j + 1, :], in_=ht_d[:, j : j + 1, :])
        ht_tiles[0] = htt0
        w1s0 = w1p.tile([128, 2, FO1], f8, name="w1s", tag="w1s")
        for q in range(4):
            fsl = slice(q * (FO1 // 4), (q + 1) * (FO1 // 4))
            nc.sync.dma_start(out=w1s0[:, 0:2, fsl], in_=w1t_d[:, 0:2, fsl])

        # ---- constants in SBUF ----
        bp_sb = {}
        for li in range(1, 6):
            fot = math.ceil(DIMS[li - 1] / 128)
            bp_sb[li] = const.tile([128, fot], f32, name=f"b{li}p", tag=f"b{li}p")
            nc.sync.dma_start(out=bp_sb[li][:], in_=bp_d[li][:])
        b6_sb = const.tile([128, 1], f32, name="b6p", tag="b6p")
        nc.sync.dma_start(out=b6_sb[:], in_=b6_d[:])
        # fp8 ones for the PE s-reduce chain, [128,2,16] so the DoubleRow
        # weight pair-step (16) meets the ISA %16 rule.  ones16m zeroes
        # history tile 0's extInfo rows.
        ones16 = const.tile([128, 2, 16], f8, name="ones16", tag="ones16")
        nc.vector.memset(ones16[:], 1.0)
        ones16m = const.tile([128, 2, 16], f8, name="ones16m", tag="ones16m")
        nc.vector.memset(ones16m[:], 1.0)
        nc.vector.memset(ones16m[0:UINFO, 0:1, :], 0.0)
        onesf = const.tile([128, 1], f32, name="onesf", tag="onesf")
        nc.vector.memset(onesf[:], 1.0)
        # per-partition mask for history tile 0 (extInfo rows excluded from cnt)
        rmask = const.tile([128, 1], f32, name="rmask", tag="rmask")
        nc.vector.memset(rmask[:], 1.0)
        nc.vector.memset(rmask[0:UINFO, :], 0.0)

        # cnt accumulator: 4 slots so one DVE op covers a whole 4-tile segment
        c_acc = const.tile([128, 4, BS], f32, name="c_acc", tag="c_acc")
        nc.vector.memset(c_acc[:, 1:4, :], 0.0)

        # s off-PE config: segments whose s-sum runs on DVE (spread out so
        # the DVE keeps pace with history arrivals) or all on GpSimd.
        s_gps = bool(cfg.get("S_GPSIMD"))
        s_dve_segs = set()
        if not s_gps and cfg.get("S_DVE_TILES", 0) > 0:
            want = cfg["S_DVE_TILES"]
            cand = [si for si in range(2, NSEG)]
            # pick evenly spaced segments until the tile budget is covered
            k = max(1, int(len(cand) * 4 / max(want, 1)))
            got = 0
            for i, si in enumerate(cand):
                if i % k == k - 1 and got < want:
                    s_dve_segs.add(si)
                    got += segs[si][1]
        s_off = s_gps or bool(s_dve_segs)
        if s_off:
            seng = nc.gpsimd if s_gps else nc.vector
            s_acc = const.tile([128, 4, BS], f32, name="s_acc", tag="s_acc")
            seng.memset(s_acc[:, 1:4, :] if s_gps else s_acc[:], 0.0)

        x1t = actp.tile([128, FO1T, BS], f8, name="x1t", tag="x1t")
        ps1 = [psp.tile([128, BS], f32, name="ps1", tag="ps") for _ in range(FO1T)]

        # ---- layer 1: stream w1 slabs against the resident history ----
        nstep = T1 // 2 + (T1 % 2)
        steps = []  # (seg_idx, lo, n) in stream order, for the PE s-reduce chain
        step_i = 0
        for si_, (ts_, tn) in enumerate(segs):
            if si_ == 0:
                htt = htt0
            else:
                htt = htp.tile([128, tn, BS], f8, name="ht", tag="ht", bufs=NSEG)
                nc.sync.dma_start(out=htt[:], in_=ht_d[:, ts_ : ts_ + tn, :])
                ht_tiles[si_] = htt

            lo = 0
            while lo < tn:
                n = 2 if lo + 2 <= tn else 1
                t = ts_ + lo
                steps.append((si_, lo, n))
                if step_i == 0:
                    w1s = w1s0
                else:
                    w1s = w1p.tile([128, 2, FO1], f8, name="w1s", tag="w1s")
                    nc.sync.dma_start(out=w1s[:, 0:n, :], in_=w1t_d[:, t : t + n, :])
                for fo in range(FO1T):
                    fsl = slice(fo * 128, (fo + 1) * 128)
                    if n == 2:
                        nc.tensor.matmul(
                            ps1[fo][:],
                            lhsT=w1s[:, 0:2, fsl],
                            rhs=htt[:, lo : lo + 2, :],
                            start=(step_i == 0),
                            stop=(step_i == nstep - 1),
                            perf_mode=mybir.MatmulPerfMode.DoubleRow,
                        )
                    else:
                        nc.tensor.matmul(
                            ps1[fo][:],
                            lhsT=w1s[:, 0, fsl],
                            rhs=htt[:, lo, :],
                            start=(step_i == 0),
                            stop=(step_i == nstep - 1),
                        )
                step_i += 1
                lo += n

            # cnt accumulation on the (otherwise idle) DVE: fused min+add,
            # one op per segment (4 accumulator slots).  Tile 0 carries the
            # extInfo rows: init slot 0 through the rmask.
            if ts_ == 0:
                nc.vector.tensor_scalar(
                    c_acc[:, 0:1, :], htt[:, 0:1, :], 1.0, rmask[:],
                    op0=ALU.min, op1=ALU.mult,
                )
                nc.vector.scalar_tensor_tensor(
                    c_acc[:, 1:2, :], htt[:, 1:2, :], 1.0, c_acc[:, 1:2, :],
                    op0=ALU.min, op1=ALU.add,
                )
            else:
                nc.vector.scalar_tensor_tensor(
                    c_acc[:, 0:tn, :], htt[:, 0:tn, :], 1.0, c_acc[:, 0:tn, :],
                    op0=ALU.min, op1=ALU.add,
                )

        # ---- s: PE ones-chain over the resident history (after layer 1
        # frees the PE; runs concurrent with the layer-1 epilogue).
        s_red = psp.tile([16, BS], f32, name="s_red", tag="ps")
        for si, (sg, lo, n) in enumerate(steps):
            htt = ht_tiles[sg]
            ones = ones16m if (sg == 0 and lo == 0) else ones16
            if n == 2:
                nc.tensor.matmul(
                    s_red[:], lhsT=ones[:], rhs=htt[:, lo : lo + 2, :],
                    start=(si == 0), stop=(si == nstep - 1),
                    perf_mode=mybir.MatmulPerfMode.DoubleRow,
                )
            else:
                nc.tensor.matmul(
                    s_red[:], lhsT=ones[:, 0, :], rhs=htt[:, lo, :],
                    start=(si == 0), stop=(si == nstep - 1),
                )

        # ---- layer-1 epilogue: alternate ACT (native Lrelu) and DVE
        # (2-op decomposition) so it drains in half the serial time.
        for fg in range(FO1T):
            if cfg.get("EPI_DVE") and fg % 2 == 1:
                tmp = scr.tile([128, BS], f32, name="tmp", tag="tmp")
                nc.vector.tensor_scalar(
                    tmp[:], ps1[fg][:], w1_unscale, bp_sb[1][:, fg : fg + 1],
                    op0=ALU.mult, op1=ALU.add,
                )
                nc.vector.scalar_tensor_tensor(
                    x1t[:, fg, :], tmp[:], SLOPE, tmp[:], op0=ALU.mult, op1=ALU.max
                )
            else:
                nc.scalar.activation(
                    x1t[:, fg, :], ps1[fg][:], AF.Lrelu,
                    bias=bp_sb[1][:, fg : fg + 1], scale=w1_unscale, alpha=SLOPE,
                )

        # ---- cnt fold + partition reduce ----
        c_fold = const.tile([128, BS], f32, name="c_fold", tag="c_fold")
        nc.vector.tensor_tensor(
            c_acc[:, 0:2, :], c_acc[:, 0:2, :], c_acc[:, 2:4, :], ALU.add
        )
        nc.vector.tensor_tensor(
            c_fold[:], c_acc[:, 0, :], c_acc[:, 1, :], ALU.add
        )
        c_red = psp.tile([1, BS], f32, name="c_red", tag="ps")
        nc.tensor.matmul(c_red[:], lhsT=onesf[:], rhs=c_fold[:], start=True, stop=True)

        # ---- meanV row: s/cnt directly in row layout (no DRAM bounce; the
        # recon matmul adds it via the constant mft row).  Staged at base
        # partition 64 so later TensorTensor ops share gen_last's base
        # partition (walrus requires equal SBUF base partitions).
        mrow = const.tile([66, 2, BS], f32, name="mrow", tag="mrow")
        nc.vector.tensor_copy(mrow[64:65, 0, :], c_red[0:1, :])
        nc.vector.tensor_copy(mrow[64:65, 1, :], s_red[0:1, :])
        nc.vector.reciprocal(mrow[64:65, 0, :], mrow[64:65, 0, :])
        nc.vector.tensor_mul(mrow[64:65, 0, :], mrow[64:65, 0, :], mrow[64:65, 1, :])

        # ---- remaining weights (emitted late so layer-1 DMA streams first) ----
        w_sb = {}
        for li in range(2, 7):
            fi, fo = DIMS[li - 2], DIMS[li - 1]
            wdt = f8 if li < 6 else bf16
            w_sb[li] = const.tile([128, fi // 128, fo], wdt, name=f"w{li}t", tag=f"w{li}t")
            nc.sync.dma_start(out=w_sb[li][:], in_=w_d[li][:])
        mft = const.tile([66, M], f8, name="mft", tag="mft")
        nc.sync.dma_start(out=mft[:], in_=mft_d[:])

        # ---- layers 2..6 + reconstruction, pipelined per 128-row batch
        # tile so output DMA starts as early as possible.
        xtiles = {1: x1t}
        for li in range(2, 6):
            fi, fo = DIMS[li - 2], DIMS[li - 1]
            xdt = f8 if li < 5 else bf16
            xtiles[li] = actp.tile([128, fo // 128, BS], xdt, name=f"x{li}t", tag=f"x{li}t")
        gen_sb = actp.tile([DIMS[5], BS], f32, name="gen", tag="gen")

        for bt in range(NBT):
            bsl = slice(bt * 128, (bt + 1) * 128)
            # MLP layers 2..5 for this batch tile
            eng = bt  # rotate engine assignment across bts
            for li in range(2, 6):
                fi, fo = DIMS[li - 2], DIMS[li - 1]
                fit, fot = fi // 128, fo // 128
                unsc = 1.0 / cfg["MLP_SCALES"][li - 2]
                xin, xout = xtiles[li - 1], xtiles[li]
                for ft in range(fot):
                    ps = psp.tile([128, 128], f32, name="ps", tag="ps")
                    ki = 0
                    while ki < fit:
                        if ki + 2 <= fit:
                            nc.tensor.matmul(
                                ps[:],
                                lhsT=w_sb[li][:, ki : ki + 2, ft * 128 : (ft + 1) * 128],
                                rhs=xin[:, ki : ki + 2, bsl],
                                start=(ki == 0),
                                stop=(ki + 2 == fit),
                                perf_mode=mybir.MatmulPerfMode.DoubleRow,
                            )
                            ki += 2
                        else:
                            nc.tensor.matmul(
                                ps[:],
                                lhsT=w_sb[li][:, ki, ft * 128 : (ft + 1) * 128],
                                rhs=xin[:, ki, bsl],
                                start=(ki == 0),
                                stop=True,
                            )
                            ki += 1
                    if cfg.get("EPI_DVE") and eng % 2 == 1:
                        tmp = scr.tile([128, 128], f32, name="tmpm", tag="tmpm")
                        nc.vector.tensor_scalar(
                            tmp[:], ps[:], unsc, bp_sb[li][:, ft : ft + 1],
                            op0=ALU.mult, op1=ALU.add,
                        )
                        nc.vector.scalar_tensor_tensor(
                            xout[:, ft, bsl], tmp[:], SLOPE, tmp[:],
                            op0=ALU.mult, op1=ALU.max,
                        )
                    else:
                        nc.scalar.activation(
                            xout[:, ft, bsl], ps[:], AF.Lrelu,
                            bias=bp_sb[li][:, ft : ft + 1], scale=unsc, alpha=SLOPE,
                        )
                    eng += 1

            # layer 6 (tanh) for this batch tile
            fi, fo = DIMS[4], DIMS[5]
            fit = fi // 128
            ps6 = psp.tile([fo, 128], f32, name="ps6", tag="ps")
            for ki in range(fit):
                nc.tensor.matmul(
                    ps6[:],
                    lhsT=w_sb[6][:, ki, 0:fo],
                    rhs=xtiles[5][:, ki, bsl],
                    start=(ki == 0),
                    stop=(ki == fit - 1),
                )
            nc.scalar.activation(
                gen_sb[:, bsl], ps6[:], AF.Tanh, bias=b6_sb[0:fo, 0:1], scale=1.0
            )

            # genext for this bt: [66, 128] bf16 — rows 0:64 factors,
            # row 64 = meanV + gen_last (added via the constant 16.0 mft
            # row 64), row 65 = 1.0 (movie_bias via mft row 65).  Row 64 is
            # written twice (memset pair, then the addv overwrite) because
            # SBUF partition offsets must be 32-aligned.
            genext = actp.tile([66, 128], bf16, name="genext", tag="genext", bufs=NBT)
            nc.vector.tensor_copy(genext[0:F, :], gen_sb[0:F, bsl])
            nc.vector.memset(genext[F : F + 2, :], 1.0)
            nc.vector.tensor_add(genext[F : F + 1, :], gen_sb[F : F + 1, bsl], mrow[64:65, 0, bsl])

            # reconstruction for this bt: out[bt*128+p, m] over movie chunks
            st = None
            for ci, (co, cw) in enumerate(CHUNKS):
                pr = psp.tile([128, cw], f32, name="pr", tag="ps")
                nc.tensor.matmul(
                    pr[:], lhsT=genext[:], rhs=mft[:, co : co + cw], start=True, stop=True
                )
                # pair two chunks per staging tile: halves the out-DMA
                # dispatch count; epilogue is a bias-free x/16 copy
                # alternating DVE/ACT.
                if ci % 2 == 0:
                    st = ost.tile([128, 2 * cfg["MCHUNK"]], odt, name="st", tag="st")
                    so, pco = 0, co
                    nc.vector.tensor_scalar(
                        st[:, so : so + cw], pr[:], 1.0 / 16.0, None, op0=ALU.mult
                    )
                else:
                    nc.scalar.activation(
                        st[:, so : so + cw], pr[:], AF.Copy, bias=0.0, scale=1.0 / 16.0
                    )
                so += cw
                if ci % 2 == 1 or ci == len(CHUNKS) - 1:
                    nc.sync.dma_start(
                        out=out_d[bsl, pco : pco + so],
                        in_=st[:, 0:so],
                    )

    nc.compile()
    return nc


def prep_in_maps(cfg, inputs):
    """Shard + lay out the full inputs into per-core DRAM input maps."""
    d = _derived(cfg)
    BS, UINFO, M, F, DIMS, T1 = cfg["BS"], cfg["UINFO"], cfg["M"], cfg["F"], cfg["DIMS"], d["T1"]
    extInfo = np.asarray(inputs["extInfo"], np.float32)
    ratings = np.asarray(inputs["ratings"], np.float32)

    # BN (eval) fold into layer 2: y = g'(lrelu1) + b' with g' = bn_g/sqrt(1+eps)
    g = np.asarray(inputs["bn_g"], np.float32) / np.float32(np.sqrt(1.0 + BN_EPS))
    bnb = np.asarray(inputs["bn_b"], np.float32)
    w2 = np.asarray(inputs["w2"], np.float32)
    w2f = w2 * g[None, :]
    b2f = np.asarray(inputs["b2"], np.float32) + w2 @ bnb

    shared = {}
    # w1t: [KH,FO1] -> padded [T1*128, FO1] -> [128, T1, FO1], fp8 pre-scaled
    w1 = np.asarray(inputs["w1"], np.float32)
    FO1 = DIMS[0]
    w1tp = np.zeros((T1 * 128, FO1), FP8)
    w1tp[0 : w1.shape[1]] = (w1.T * np.float32(cfg["W1_SCALE"])).astype(FP8)
    shared["w1t"] = np.ascontiguousarray(w1tp.reshape(T1, 128, FO1).transpose(1, 0, 2))

    def pack_w(wT, fo, dt=BF16, scale=1.0):
        fi = wT.shape[0]
        w = (wT.astype(np.float32) * np.float32(scale)).astype(dt)
        return np.ascontiguousarray(w.reshape(fi // 128, 128, fo).transpose(1, 0, 2))

    scs = cfg["MLP_SCALES"]
    shared["w2t"] = pack_w(w2f.T, DIMS[1], FP8, scs[0])
    for li, wname in ((3, "w3"), (4, "w4"), (5, "w5"), (6, "w6")):
        w = np.asarray(inputs[wname], np.float32)
        fo = DIMS[li - 1]
        if li < 6:
            shared[f"w{li}t"] = pack_w(w.T, fo, FP8, scs[li - 2])
        else:
            shared[f"w{li}t"] = pack_w(w.T, fo)

    def pack_b(b, fo):
        fot = math.ceil(fo / 128)
        bp = np.zeros(fot * 128, np.float32)
        bp[:fo] = b
        return np.ascontiguousarray(bp.reshape(fot, 128).T)

    bsrc = {1: np.asarray(inputs["b1"], np.float32), 2: b2f}
    for li in (3, 4, 5):
        bsrc[li] = np.asarray(inputs[f"b{li}"], np.float32)
    for li in range(1, 6):
        shared[f"b{li}p"] = pack_b(bsrc[li], DIMS[li - 1])
    shared["b6p"] = pack_b(np.asarray(inputs["b6"], np.float32), DIMS[5])

    # fp8 mft [66, M]: rows 0:64 = 16*factors.T, row 64 = 16.0 (carries
    # meanV+gen_last from genext row 64), row 65 = 16*movie_bias; the recon
    # epilogue divides the matmul result back down by 16.
    mft = np.zeros((66, M), FP8)
    mft[0:F] = (np.asarray(inputs["movie_factors"], np.float32).T * np.float32(16.0)).astype(FP8)
    mft[F] = np.float32(16.0)
    mft[F + 1] = (np.asarray(inputs["movie_bias"], np.float32) * np.float32(16.0)).astype(FP8)
    shared["mft"] = mft

    in_maps = []
    for c in range(NCORES):
        sl = slice(c * BS, (c + 1) * BS)
        htc = np.zeros((T1 * 128, BS), FP8)
        htc[0:UINFO] = extInfo[sl].T.astype(FP8)
        htc[UINFO : UINFO + M] = ratings[sl].T.astype(FP8)
        m = dict(shared)
        m["ht"] = np.ascontiguousarray(htc.reshape(T1, 128, BS).transpose(1, 0, 2))
        in_maps.append(m)
    return in_maps


_NC_CACHE = {}


def run_on_hw(cfg, inputs, trace=False):
    from concourse.bass_utils import run_bass_kernel_spmd

    key = tuple(sorted((k, v) for k, v in cfg.items() if not isinstance(v, tuple))) + (
        cfg["DIMS"],
        cfg["MLP_SCALES"],
    )
    if key not in _NC_CACHE:
        _NC_CACHE[key] = build_nc(cfg)
    nc = _NC_CACHE[key]
    in_maps = prep_in_maps(cfg, inputs)
    br = run_bass_kernel_spmd(nc, in_maps, list(range(NCORES)), trace=trace)
    BS, M = cfg["BS"], cfg["M"]
    out = np.empty((NCORES * BS, M), np.float32)
    for c in range(NCORES):
        out[c * BS : (c + 1) * BS] = np.asarray(br.results[c]["out"], dtype=np.float32)
    return out, br


def kernel(**inputs) -> np.ndarray:
    try:
        out, _ = run_on_hw(FULL_CFG, inputs, trace=False)
    except Exception:
        # one retry for transient device/runtime hiccups
        out, _ = run_on_hw(FULL_CFG, inputs, trace=False)
    return out


# revision 19
# speedup vs baseline: 1.0767x; 1.0767x over previous
"""Trainium2 Bass kernel for nn_DLFG_79817672229311 (segment_reduce).

Computes, data-parallel over the batch axis on 8 NeuronCores:
  history = [extInfo, ratings]                       # [BS, 20032] per core
  x1 = lrelu(history @ w1.T + b1); BN eval           # BN folded into w2 on host
  x2..x5 = lrelu(x @ wl.T + bl)
  gen = tanh(x5 @ w6.T + b6)                         # [BS, 65]
  s, cnt = per-row sum / count of nonzero ratings
  meanV = s / cnt                                    # cnt >= MIN_CNT always holds here
  out = gen[:, :64] @ movie_factors.T + (meanV + gen[:, 64])[:, None] + movie_bias

Design (per core), v2:
- Activations ride transposed ([feature, batch]): batch (512) is the matmul
  free dim, features the partition dim, so no on-device transposes are needed.
- History is staged to SBUF once in fp8 (ratings 0..5 are exact in e4m3) and
  layer 1 runs fp8 DoubleRow against 2^15-pre-scaled fp8 w1 slabs streamed
  from HBM; layers 2-5 are fp8 DoubleRow with per-layer pre-scales; layer 6
  stays bf16.  LeakyReLU is the native Lrelu ACT op, alternated with a 2-op
  DVE decomposition so the layer-1 epilogue never serializes on one engine.
- A chain of dummy matmuls at t=0 ramps the PE out of its low p-state while
  the first history/w1 DMAs (issued before everything else, split across
  queues) are still in flight.
- s is a PE ones-matmul chain over the resident history (DoubleRow,
  [128,2,16] ones to satisfy the pair-step%16 ISA rule); cnt is a 4-wide
  fused min+add chain on the otherwise-idle Vector engine.  Both finish
  under the layer-1 shadow.
- meanV + gen_last is folded into the reconstruction matmul itself: an extra
  genext row carries addv against a constant 16.0 row in mft, so the recon
  epilogue is a bias-free x/16 copy (alternating DVE/ACT) and no transpose
  bounce through DRAM is needed.  movie_factors ride fp8 x16 in a 66-row
  tile; output is written fp16 (host upcasts).
- The MLP and reconstruction are pipelined per 128-row batch tile so output
  DMA (the tail phase's real floor) starts right after layer 1 + s-chain.
"""

import math
import sys

sys.path.insert(0, "/opt/trn_rl_repo")

import numpy as np
import ml_dtypes

BF16 = ml_dtypes.bfloat16
FP8 = ml_dtypes.float8_e4m3

NCORES = 8
BN_EPS = 0.05
SLOPE = 0.01

FULL_CFG = dict(
    BS=512,  # per-core batch
    UINFO=32,
    M=20000,
    F=64,
    DIMS=(1024, 512, 256, 512, 1024, 65),  # fan-outs of the 6 linear layers
    HTC=4,  # history K-tiles per DMA chunk (must be even for DoubleRow pairs)
    MCHUNK=512,  # reconstruction movie-axis chunk (one PSUM bank)
    W1_SCALE=2.0**15,  # fp8 pre-scale: w1 ~ U(+-0.007) sits in e4m3 subnormals
    OUT_FP16=True,  # write output fp16 (host upcasts) to halve the drain DMA
    MLP_SCALES=(4096.0, 4096.0, 2048.0, 4096.0),  # 2^k per layer, keep |w|*s under e4m3 max 240
    WARMUP_N=44,  # dummy matmuls to ramp the PE p-state during initial DMA wait
    OST_BUFS=12,  # recon output staging depth
    EPI_DVE=True,  # alternate layer epilogues between ACT and DVE
    S_DVE_TILES=0,  # history tiles whose s-sum runs on DVE instead of the PE
    S_GPSIMD=False,  # run the whole s-chain on GpSimd (frees the PE entirely)
    MFT_PAD128=False,  # pad recon contraction to 128 partitions (shape probe)
    RECON_EPI="alt",  # recon epilogue engine: alt | act | vec
    MLP_FULLB=False,  # run layers 2..6 full-batch instead of per batch tile
    FILLER_N=0,  # zero-contributing filler matmuls per post-l1 PSUM group:
    # they keep the PE's HAM clock gate at 8/8 (2.4 GHz) through the
    # drain-paced recon phase without extra PSUM banks (they join the
    # group with start=True and add exact zeros from the warm tile)
)


def _derived(cfg):
    d = dict(cfg)
    d["KH"] = cfg["UINFO"] + cfg["M"]
    d["T1"] = math.ceil(d["KH"] / 128)  # history K tiles (padded)
    d["NBT"] = cfg["BS"] // 128  # batch tiles per core
    mch = cfg["MCHUNK"]
    d["CHUNKS"] = [(o, min(mch, cfg["M"] - o)) for o in range(0, cfg["M"], mch)]
    d["NHC"] = math.ceil(d["T1"] / cfg["HTC"])  # history DMA chunks
    return d


def build_nc(cfg):
    """Build + compile the (single-core SPMD) Bass program."""
    import concourse.bass as bass
    import concourse.tile as tile
    from concourse import bacc, mybir

    d = _derived(cfg)
    BS, UINFO, M, F = cfg["BS"], cfg["UINFO"], cfg["M"], cfg["F"]
    DIMS = cfg["DIMS"]
    T1, NBT, CHUNKS, HTC = d["T1"], d["NBT"], d["CHUNKS"], cfg["HTC"]
    FO1 = DIMS[0]
    FO1T = FO1 // 128
    w1_unscale = 1.0 / cfg["W1_SCALE"]
    f32 = mybir.dt.float32
    bf16 = mybir.dt.bfloat16
    f16 = mybir.dt.float16
    f8 = mybir.dt.float8e4
    AF = mybir.ActivationFunctionType
    ALU = mybir.AluOpType

    nc = bacc.Bacc("TRN2", target_bir_lowering=False, debug=False)

    # ---- DRAM I/O ----
    ht_d = nc.dram_tensor("ht", [128, T1, BS], f8, kind="ExternalInput")
    w1t_d = nc.dram_tensor("w1t", [128, T1, FO1], f8, kind="ExternalInput")
    w_d = {}
    for li in range(2, 7):
        fi, fo = DIMS[li - 2], DIMS[li - 1]
        wdt = f8 if li < 6 else bf16
        w_d[li] = nc.dram_tensor(f"w{li}t", [128, fi // 128, fo], wdt, kind="ExternalInput")
    bp_d = {}
    for li in range(1, 6):
        fot = math.ceil(DIMS[li - 1] / 128)
        bp_d[li] = nc.dram_tensor(f"b{li}p", [128, fot], f32, kind="ExternalInput")
    b6_d = nc.dram_tensor("b6p", [128, 1], f32, kind="ExternalInput")
    MROWS = 128 if cfg.get("MFT_PAD128") else 66
    mft_d = nc.dram_tensor("mft", [MROWS, M], f8, kind="ExternalInput")
    odt = f16 if cfg.get("OUT_FP16") else bf16
    out_d = nc.dram_tensor("out", [BS, M], odt, kind="ExternalOutput")

    # layer-1 history segments: small warmup segs so the first matmuls
    # start ASAP, then HTC-sized chunks
    segs = []
    t0 = 0
    for tn in ([2, 2] if T1 > 4 else []):
        segs.append((t0, tn))
        t0 += tn
    while t0 < T1:
        tn = min(HTC, T1 - t0)
        segs.append((t0, tn))
        t0 += tn
    NSEG = len(segs)

    with tile.TileContext(nc) as tc, bass.ExitStack() as ctx:
        const = ctx.enter_context(tc.tile_pool(name="const", bufs=1))
        htp = ctx.enter_context(tc.tile_pool(name="htp", bufs=1))
        w1p = ctx.enter_context(tc.tile_pool(name="w1p", bufs=8))
        actp = ctx.enter_context(tc.tile_pool(name="actp", bufs=1))
        scr = ctx.enter_context(tc.tile_pool(name="scr", bufs=4))
        ost = ctx.enter_context(tc.tile_pool(name="ost", bufs=cfg["OST_BUFS"]))
        psp = ctx.enter_context(tc.tile_pool(name="psp", bufs=8, space="PSUM"))

        # ---- PE p-state warmup: dummy matmuls on a zeroed tile while the
        # first real DMAs land.  The TRN2 PE ramps 0.65->1.2->2.4 GHz with
        # ~3us of continuous execution; without this the first ~4 layer-1
        # steps run at less than half speed.
        warm = const.tile([128, 128], f8, name="warm", tag="warm")
        nc.vector.memset(warm[:], 0.0)
        wps = psp.tile([128, 128], f32, name="wps", tag="ps")
        for _ in range(cfg["WARMUP_N"]):
            nc.tensor.matmul(wps[:], lhsT=warm[:], rhs=warm[:], start=True, stop=True)

        # ---- first history segment + first w1 slab, issued before all other
        # DMAs and split across queues so the first real matmul fires early.
        ht_tiles = {}
        ts0, tn0 = segs[0]
        htt0 = htp.tile([128, tn0, BS], f8, name="ht", tag="ht", bufs=NSEG)
        for j in range(tn0):
            nc.sync.dma_start(out=htt0[:, j : j + 1, :], in_=ht_d[:, j : j + 1, :])
        ht_tiles[0] = htt0
        w1s0 = w1p.tile([128, 2, FO1], f8, name="w1s", tag="w1s")
        for q in range(4):
            fsl = slice(q * (FO1 // 4), (q + 1) * (FO1 // 4))
            nc.sync.dma_start(out=w1s0[:, 0:2, fsl], in_=w1t_d[:, 0:2, fsl])

        # ---- constants in SBUF ----
        bp_sb = {}
        for li in range(1, 6):
            fot = math.ceil(DIMS[li - 1] / 128)
            bp_sb[li] = const.tile([128, fot], f32, name=f"b{li}p", tag=f"b{li}p")
            nc.sync.dma_start(out=bp_sb[li][:], in_=bp_d[li][:])
        b6_sb = const.tile([128, 1], f32, name="b6p", tag="b6p")
        nc.sync.dma_start(out=b6_sb[:], in_=b6_d[:])
        # fp8 ones for the PE s-reduce chain, [128,2,16] so the DoubleRow
        # weight pair-step (16) meets the ISA %16 rule.  ones16m zeroes
        # history tile 0's extInfo rows.
        ones16 = const.tile([128, 2, 16], f8, name="ones16", tag="ones16")
        nc.vector.memset(ones16[:], 1.0)
        ones16m = const.tile([128, 2, 16], f8, name="ones16m", tag="ones16m")
        nc.vector.memset(ones16m[:], 1.0)
        nc.vector.memset(ones16m[0:UINFO, 0:1, :], 0.0)
        onesf = const.tile([128, 1], f32, name="onesf", tag="onesf")
        nc.vector.memset(onesf[:], 1.0)
        # per-partition mask for history tile 0 (extInfo rows excluded from cnt)
        rmask = const.tile([128, 1], f32, name="rmask", tag="rmask")
        nc.vector.memset(rmask[:], 1.0)
        nc.vector.memset(rmask[0:UINFO, :], 0.0)

        # cnt accumulator: 4 slots so one DVE op covers a whole 4-tile segment
        c_acc = const.tile([128, 4, BS], f32, name="c_acc", tag="c_acc")
        nc.vector.memset(c_acc[:, 1:4, :], 0.0)

        # s off-PE config: segments whose s-sum runs on DVE (spread out so
        # the DVE keeps pace with history arrivals) or all on GpSimd.
        s_gps = bool(cfg.get("S_GPSIMD"))
        s_dve_segs = set()
        if not s_gps and cfg.get("S_DVE_TILES", 0) > 0:
            want = cfg["S_DVE_TILES"]
            cand = [si for si in range(2, NSEG)]
            # pick evenly spaced segments until the tile budget is covered
            k = max(1, int(len(cand) * 4 / max(want, 1)))
            got = 0
            for i, si in enumerate(cand):
                if i % k == k - 1 and got < want:
                    s_dve_segs.add(si)
                    got += segs[si][1]
        s_off = s_gps or bool(s_dve_segs)
        if s_off:
            seng = nc.gpsimd if s_gps else nc.vector
            s_acc = const.tile([128, 4, BS], f32, name="s_acc", tag="s_acc")
            seng.memset(s_acc[:, 1:4, :] if s_gps else s_acc[:], 0.0)

        x1t = actp.tile([128, FO1T, BS], f8, name="x1t", tag="x1t")
        ps1 = [psp.tile([128, BS], f32, name="ps1", tag="ps") for _ in range(FO1T)]

        # ---- layer 1: stream w1 slabs against the resident history ----
        nstep = T1 // 2 + (T1 % 2)
        steps = []  # (seg_idx, lo, n) in stream order, for the PE s-reduce chain
        step_i = 0
        for si_, (ts_, tn) in enumerate(segs):
            if si_ == 0:
                htt = htt0
            else:
                htt = htp.tile([128, tn, BS], f8, name="ht", tag="ht", bufs=NSEG)
                nc.sync.dma_start(out=htt[:], in_=ht_d[:, ts_ : ts_ + tn, :])
                ht_tiles[si_] = htt

            lo = 0
            while lo < tn:
                n = 2 if lo + 2 <= tn else 1
                t = ts_ + lo
                steps.append((si_, lo, n))
                if step_i == 0:
                    w1s = w1s0
                else:
                    w1s = w1p.tile([128, 2, FO1], f8, name="w1s", tag="w1s")
                    nc.sync.dma_start(out=w1s[:, 0:n, :], in_=w1t_d[:, t : t + n, :])
                for fo in range(FO1T):
                    fsl = slice(fo * 128, (fo + 1) * 128)
                    if n == 2:
                        nc.tensor.matmul(
                            ps1[fo][:],
                            lhsT=w1s[:, 0:2, fsl],
                            rhs=htt[:, lo : lo + 2, :],
                            start=(step_i == 0),
                            stop=(step_i == nstep - 1),
                            perf_mode=mybir.MatmulPerfMode.DoubleRow,
                        )
                    else:
                        nc.tensor.matmul(
                            ps1[fo][:],
                            lhsT=w1s[:, 0, fsl],
                            rhs=htt[:, lo, :],
                            start=(step_i == 0),
                            stop=(step_i == nstep - 1),
                        )
                step_i += 1
                lo += n

            # cnt accumulation on the (otherwise idle) DVE: fused min+add,
            # one op per segment (4 accumulator slots).  Tile 0 carries the
            # extInfo rows: init slot 0 through the rmask.
            if ts_ == 0:
                nc.vector.tensor_scalar(
                    c_acc[:, 0:1, :], htt[:, 0:1, :], 1.0, rmask[:],
                    op0=ALU.min, op1=ALU.mult,
                )
                nc.vector.scalar_tensor_tensor(
                    c_acc[:, 1:2, :], htt[:, 1:2, :], 1.0, c_acc[:, 1:2, :],
                    op0=ALU.min, op1=ALU.add,
                )
            else:
                nc.vector.scalar_tensor_tensor(
                    c_acc[:, 0:tn, :], htt[:, 0:tn, :], 1.0, c_acc[:, 0:tn, :],
                    op0=ALU.min, op1=ALU.add,
                )

        # ---- s: PE ones-chain over the resident history (after layer 1
        # frees the PE; runs concurrent with the layer-1 epilogue).
        s_red = psp.tile([16, BS], f32, name="s_red", tag="ps")
        for si, (sg, lo, n) in enumerate(steps):
            htt = ht_tiles[sg]
            ones = ones16m if (sg == 0 and lo == 0) else ones16
            if n == 2:
                nc.tensor.matmul(
                    s_red[:], lhsT=ones[:], rhs=htt[:, lo : lo + 2, :],
                    start=(si == 0), stop=(si == nstep - 1),
                    perf_mode=mybir.MatmulPerfMode.DoubleRow,
                )
            else:
                nc.tensor.matmul(
                    s_red[:], lhsT=ones[:, 0, :], rhs=htt[:, lo, :],
                    start=(si == 0), stop=(si == nstep - 1),
                )

        # ---- layer-1 epilogue: alternate ACT (native Lrelu) and DVE
        # (2-op decomposition) so it drains in half the serial time.
        for fg in range(FO1T):
            if cfg.get("EPI_DVE") and fg % 2 == 1:
                tmp = scr.tile([128, BS], f32, name="tmp", tag="tmp")
                nc.vector.tensor_scalar(
                    tmp[:], ps1[fg][:], w1_unscale, bp_sb[1][:, fg : fg + 1],
                    op0=ALU.mult, op1=ALU.add,
                )
                nc.vector.scalar_tensor_tensor(
                    x1t[:, fg, :], tmp[:], SLOPE, tmp[:], op0=ALU.mult, op1=ALU.max
                )
            else:
                nc.scalar.activation(
                    x1t[:, fg, :], ps1[fg][:], AF.Lrelu,
                    bias=bp_sb[1][:, fg : fg + 1], scale=w1_unscale, alpha=SLOPE,
                )

        # ---- cnt fold + partition reduce ----
        c_fold = const.tile([128, BS], f32, name="c_fold", tag="c_fold")
        nc.vector.tensor_tensor(
            c_acc[:, 0:2, :], c_acc[:, 0:2, :], c_acc[:, 2:4, :], ALU.add
        )
        nc.vector.tensor_tensor(
            c_fold[:], c_acc[:, 0, :], c_acc[:, 1, :], ALU.add
        )
        c_red = psp.tile([1, BS], f32, name="c_red", tag="ps")
        nc.tensor.matmul(c_red[:], lhsT=onesf[:], rhs=c_fold[:], start=True, stop=True)

        # ---- meanV row: s/cnt directly in row layout (no DRAM bounce; the
        # recon matmul adds it via the constant mft row).  Staged at base
        # partition 64 so later TensorTensor ops share gen_last's base
        # partition (walrus requires equal SBUF base partitions).
        mrow = const.tile([66, 2, BS], f32, name="mrow", tag="mrow")
        nc.vector.tensor_copy(mrow[64:65, 0, :], c_red[0:1, :])
        nc.vector.tensor_copy(mrow[64:65, 1, :], s_red[0:1, :])
        nc.vector.reciprocal(mrow[64:65, 0, :], mrow[64:65, 0, :])
        nc.vector.tensor_mul(mrow[64:65, 0, :], mrow[64:65, 0, :], mrow[64:65, 1, :])

        # ---- remaining weights (emitted late so layer-1 DMA streams first) ----
        w_sb = {}
        for li in range(2, 7):
            fi, fo = DIMS[li - 2], DIMS[li - 1]
            wdt = f8 if li < 6 else bf16
            w_sb[li] = const.tile([128, fi // 128, fo], wdt, name=f"w{li}t", tag=f"w{li}t")
            nc.sync.dma_start(out=w_sb[li][:], in_=w_d[li][:])
        mft = const.tile([MROWS, M], f8, name="mft", tag="mft")
        nc.sync.dma_start(out=mft[:], in_=mft_d[:])

        # ---- layers 2..6 + reconstruction, pipelined per 128-row batch
        # tile so output DMA starts as early as possible.
        xtiles = {1: x1t}
        for li in range(2, 6):
            fi, fo = DIMS[li - 2], DIMS[li - 1]
            xdt = f8 if li < 5 else bf16
            xtiles[li] = actp.tile([128, fo // 128, BS], xdt, name=f"x{li}t", tag=f"x{li}t")
        gen_sb = actp.tile([DIMS[5], BS], f32, name="gen", tag="gen")

        def mlp_body(bsl, w):
            # MLP layers 2..5 then layer 6 (tanh) for batch slice bsl
            eng = 0
            for li in range(2, 6):
                fi, fo = DIMS[li - 2], DIMS[li - 1]
                fit, fot = fi // 128, fo // 128
                unsc = 1.0 / cfg["MLP_SCALES"][li - 2]
                xin, xout = xtiles[li - 1], xtiles[li]
                for ft in range(fot):
                    ps = psp.tile([128, w], f32, name="ps", tag="ps")
                    ki = 0
                    while ki < fit:
                        if ki + 2 <= fit:
                            nc.tensor.matmul(
                                ps[:],
                                lhsT=w_sb[li][:, ki : ki + 2, ft * 128 : (ft + 1) * 128],
                                rhs=xin[:, ki : ki + 2, bsl],
                                start=(ki == 0),
                                stop=(ki + 2 == fit),
                                perf_mode=mybir.MatmulPerfMode.DoubleRow,
                            )
                            ki += 2
                        else:
                            nc.tensor.matmul(
                                ps[:],
                                lhsT=w_sb[li][:, ki, ft * 128 : (ft + 1) * 128],
                                rhs=xin[:, ki, bsl],
                                start=(ki == 0),
                                stop=True,
                            )
                            ki += 1
                    if cfg.get("EPI_DVE") and eng % 2 == 1:
                        tmp = scr.tile([128, w], f32, name="tmpm", tag="tmpm")
                        nc.vector.tensor_scalar(
                            tmp[:], ps[:], unsc, bp_sb[li][:, ft : ft + 1],
                            op0=ALU.mult, op1=ALU.add,
                        )
                        nc.vector.scalar_tensor_tensor(
                            xout[:, ft, bsl], tmp[:], SLOPE, tmp[:],
                            op0=ALU.mult, op1=ALU.max,
                        )
                    else:
                        nc.scalar.activation(
                            xout[:, ft, bsl], ps[:], AF.Lrelu,
                            bias=bp_sb[li][:, ft : ft + 1], scale=unsc, alpha=SLOPE,
                        )
                    eng += 1

            fi, fo = DIMS[4], DIMS[5]
            fit = fi // 128
            ps6 = psp.tile([fo, w], f32, name="ps6", tag="ps")
            for ki in range(fit):
                nc.tensor.matmul(
                    ps6[:],
                    lhsT=w_sb[6][:, ki, 0:fo],
                    rhs=xtiles[5][:, ki, bsl],
                    start=(ki == 0),
                    stop=(ki == fit - 1),
                )
            nc.scalar.activation(
                gen_sb[:, bsl], ps6[:], AF.Tanh, bias=b6_sb[0:fo, 0:1], scale=1.0
            )

        if cfg.get("MLP_FULLB"):
            mlp_body(slice(0, BS), BS)

        for bt in range(NBT):
            bsl = slice(bt * 128, (bt + 1) * 128)
            if not cfg.get("MLP_FULLB"):
                mlp_body(bsl, 128)

            # genext for this bt: [MROWS, 128] bf16 — rows 0:64 factors,
            # row 64 = meanV + gen_last (added via the constant 16.0 mft
            # row 64), row 65 = 1.0 (movie_bias via mft row 65).  Row 64 is
            # written twice (memset pair, then the addv overwrite) because
            # SBUF partition offsets must be 32-aligned.
            genext = actp.tile([MROWS, 128], bf16, name="genext", tag="genext", bufs=NBT)
            nc.vector.tensor_copy(genext[0:F, :], gen_sb[0:F, bsl])
            if MROWS == 128:
                nc.vector.memset(genext[F : 128, :], 0.0)
            nc.vector.memset(genext[F : F + 2, :], 1.0)
            nc.vector.tensor_add(genext[F : F + 1, :], gen_sb[F : F + 1, bsl], mrow[64:65, 0, bsl])

            # reconstruction for this bt: out[bt*128+p, m] over movie chunks
            st = None
            for ci, (co, cw) in enumerate(CHUNKS):
                pr = psp.tile([128, cw], f32, name="pr", tag="ps")
                nc.tensor.matmul(
                    pr[:], lhsT=genext[:], rhs=mft[:, co : co + cw], start=True, stop=True
                )
                # pair two chunks per staging tile: halves the out-DMA
                # dispatch count; epilogue is a bias-free x/16 copy
                # alternating DVE/ACT.
                if ci % 2 == 0:
                    st = ost.tile([128, 2 * cfg["MCHUNK"]], odt, name="st", tag="st")
                    so, pco = 0, co
                emode = cfg.get("RECON_EPI", "alt")
                use_vec = emode == "vec" or (emode == "alt" and ci % 2 == 0)
                if use_vec:
                    nc.vector.tensor_scalar(
                        st[:, so : so + cw], pr[:], 1.0 / 16.0, None, op0=ALU.mult
                    )
                else:
                    nc.scalar.activation(
                        st[:, so : so + cw], pr[:], AF.Copy, bias=0.0, scale=1.0 / 16.0
                    )
                so += cw
                if ci % 2 == 1 or ci == len(CHUNKS) - 1:
                    nc.sync.dma_start(
                        out=out_d[bsl, pco : pco + so],
                        in_=st[:, 0:so],
                    )

    nc.compile()
    return nc


def prep_in_maps(cfg, inputs):
    """Shard + lay out the full inputs into per-core DRAM input maps."""
    d = _derived(cfg)
    BS, UINFO, M, F, DIMS, T1 = cfg["BS"], cfg["UINFO"], cfg["M"], cfg["F"], cfg["DIMS"], d["T1"]
    extInfo = np.asarray(inputs["extInfo"], np.float32)
    ratings = np.asarray(inputs["ratings"], np.float32)

    # BN (eval) fold into layer 2: y = g'(lrelu1) + b' with g' = bn_g/sqrt(1+eps)
    g = np.asarray(inputs["bn_g"], np.float32) / np.float32(np.sqrt(1.0 + BN_EPS))
    bnb = np.asarray(inputs["bn_b"], np.float32)
    w2 = np.asarray(inputs["w2"], np.float32)
    w2f = w2 * g[None, :]
    b2f = np.asarray(inputs["b2"], np.float32) + w2 @ bnb

    shared = {}
    # w1t: [KH,FO1] -> padded [T1*128, FO1] -> [128, T1, FO1], fp8 pre-scaled
    w1 = np.asarray(inputs["w1"], np.float32)
    FO1 = DIMS[0]
    w1tp = np.zeros((T1 * 128, FO1), FP8)
    w1tp[0 : w1.shape[1]] = (w1.T * np.float32(cfg["W1_SCALE"])).astype(FP8)
    shared["w1t"] = np.ascontiguousarray(w1tp.reshape(T1, 128, FO1).transpose(1, 0, 2))

    def pack_w(wT, fo, dt=BF16, scale=1.0):
        fi = wT.shape[0]
        w = (wT.astype(np.float32) * np.float32(scale)).astype(dt)
        return np.ascontiguousarray(w.reshape(fi // 128, 128, fo).transpose(1, 0, 2))

    scs = cfg["MLP_SCALES"]
    shared["w2t"] = pack_w(w2f.T, DIMS[1], FP8, scs[0])
    for li, wname in ((3, "w3"), (4, "w4"), (5, "w5"), (6, "w6")):
        w = np.asarray(inputs[wname], np.float32)
        fo = DIMS[li - 1]
        if li < 6:
            shared[f"w{li}t"] = pack_w(w.T, fo, FP8, scs[li - 2])
        else:
            shared[f"w{li}t"] = pack_w(w.T, fo)

    def pack_b(b, fo):
        fot = math.ceil(fo / 128)
        bp = np.zeros(fot * 128, np.float32)
        bp[:fo] = b
        return np.ascontiguousarray(bp.reshape(fot, 128).T)

    bsrc = {1: np.asarray(inputs["b1"], np.float32), 2: b2f}
    for li in (3, 4, 5):
        bsrc[li] = np.asarray(inputs[f"b{li}"], np.float32)
    for li in range(1, 6):
        shared[f"b{li}p"] = pack_b(bsrc[li], DIMS[li - 1])
    shared["b6p"] = pack_b(np.asarray(inputs["b6"], np.float32), DIMS[5])

    # fp8 mft [66, M]: rows 0:64 = 16*factors.T, row 64 = 16.0 (carries
    # meanV+gen_last from genext row 64), row 65 = 16*movie_bias; the recon
    # epilogue divides the matmul result back down by 16.
    mft = np.zeros((128 if cfg.get("MFT_PAD128") else 66, M), FP8)
    mft[0:F] = (np.asarray(inputs["movie_factors"], np.float32).T * np.float32(16.0)).astype(FP8)
    mft[F] = np.float32(16.0)
    mft[F + 1] = (np.asarray(inputs["movie_bias"], np.float32) * np.float32(16.0)).astype(FP8)
    shared["mft"] = mft

    in_maps = []
    for c in range(NCORES):
        sl = slice(c * BS, (c + 1) * BS)
        htc = np.zeros((T1 * 128, BS), FP8)
        htc[0:UINFO] = extInfo[sl].T.astype(FP8)
        htc[UINFO : UINFO + M] = ratings[sl].T.astype(FP8)
        m = dict(shared)
        m["ht"] = np.ascontiguousarray(htc.reshape(T1, 128, BS).transpose(1, 0, 2))
        in_maps.append(m)
    return in_maps


_NC_CACHE = {}


def run_on_hw(cfg, inputs, trace=False):
    from concourse.bass_utils import run_bass_kernel_spmd

    key = tuple(sorted((k, v) for k, v in cfg.items() if not isinstance(v, tuple))) + (
        cfg["DIMS"],
        cfg["MLP_SCALES"],
    )
    if key not in _NC_CACHE:
        _NC_CACHE[key] = build_nc(cfg)
    nc = _NC_CACHE[key]
    in_maps = prep_in_maps(cfg, inputs)
    br = run_bass_kernel_spmd(nc, in_maps, list(range(NCORES)), trace=trace)
    BS, M = cfg["BS"], cfg["M"]
    out = np.empty((NCORES * BS, M), np.float32)
    for c in range(NCORES):
        out[c * BS : (c + 1) * BS] = np.asarray(br.results[c]["out"], dtype=np.float32)
    return out, br


def kernel(**inputs) -> np.ndarray:
    try:
        out, _ = run_on_hw(FULL_CFG, inputs, trace=False)
    except Exception:
        # one retry for transient device/runtime hiccups
        out, _ = run_on_hw(FULL_CFG, inputs, trace=False)
    return out
